# revision 42
# baseline (speedup 1.0000x reference)
"""Sparse-attention Bass kernel for 8 TRN2 NeuronCores.

Sharding: (batch, query-quarter) parallel. Core c owns batch b=c//4 and query
rows [q*1024, (q+1)*1024) of that batch (q=c%4), processed as two 512-query
passes (ic=0,1). K/V for the core's batch are computed once and reused by
both passes (half the redundant QKV work of pure query-sharding).

Elementwise strategy (the naive version is exp/mask-bound on ACT+DVE):
  * Q weights are pre-scaled on host by A = 2^7*log2(e)*scale, so PSUM scores
    arrive as s' with exp(s*scale) = 2^(s'/128).
  * Route R2 (DVE): one fused scalar_tensor_tensor per tile computes
    (s' + B) * mask -> int16 (Schraudolph bitcast: int16 pattern read as bf16
    is the masked exp). One DVE op replaces ACT-exp + DVE-mask.
  * Route R3 (PE+ACT): a diagonal matmul injects +BIG*mask into the score
    PSUM (eye*BIG as fp8e5 lhsT, mask fp8e5 rhs), then one ACT exp with
    bias=-BIG*ln2/128 finishes the tile (exact fp32 cancellation for kept
    elements, exp(-177) == 0 for masked ones). No DVE op at all.
  R2/R3 alternate on "light" slots (no deferred-QKV PE work) so ACT, DVE and
  PE all pipeline; the Schraudolph constant C is centered (-6.25) so the
  trick's mean inflation (2.82%) cancels against the true-exp route.

st PSUM pool at bufs=3 is critical: it gives PE enough runway that HAM never
re-throttles the clock to 1.2 GHz (measured 792us -> 25us cold time).
"""

import numpy as np
from contextlib import ExitStack

import concourse.bass as bass
import concourse.tile as tile
from concourse import bacc, mybir
from concourse.bass_utils import run_bass_kernel_spmd

BF16 = mybir.dt.bfloat16
F32 = mybir.dt.float32
I16 = mybir.dt.int16
FP8 = mybir.dt.float8e5
NPBF16 = mybir.dt.np(BF16)
NPFP8 = mybir.dt.np(FP8)

B, N, DIM, H, D = 2, 4096, 512, 16, 32
NCORES = 8
NQQ = 1024                  # query rows per core (one batch elem)
NQ = 512                    # query rows per pass
G = 4                       # head groups (4 heads each)
HG = H // G                 # heads per group (4)
JB = N // 128               # key blocks (32)
SCALE = float(D) ** -0.5

# Schraudolph / exp constants
A_PRE = 128.0 * np.log2(np.e) * SCALE          # folded into Q weights on host
C_CENTER = -6.25                                # centers trick vs true exp
B_CONST = float(127.0 * 128.0 + C_CENTER)       # STT additive constant
SC = float(np.float32(np.log(2.0) / 128.0))     # ACT exp scale (undoes A_PRE)
BIG = 32768.0                                   # mask injection magnitude
BIAS_NEG = float(-(np.float32(SC) * np.float32(BIG)))  # exact fp32 cancel

_CACHE = {}


class _nullctx:
    def __enter__(self):
        return None

    def __exit__(self, *a):
        return False


def _deferred(ic, g, h2, jb):
    """QKV work hidden under ic=0 attention. Returns list of call specs."""
    if ic != 0:
        return []
    calls = []
    if g == 0 and h2 == 0 and jb < JB - 1:
        calls.append(("v", jb + 1))
    if h2 == 1 and g in (0, 1) and jb < 8:
        calls.append(("kt", 2 * g + 1, jb))          # kt1 (g0), kt3 (g1)
    if h2 == 0 and g == 1 and jb < 8:
        calls.append(("kt", 2, jb))                  # kt2
    if jb == 10 and (g, h2) in ((0, 1), (1, 0), (1, 1)):
        calls.append(("qt", 2 * g + h2, 0))          # qt1, qt2, qt3 for ic0
    if jb == 2 and g >= 2:
        calls.append(("qt", 2 * (g - 2) + h2, 1))    # qt0..qt3 for ic1
    return calls


def build_nc():
    nc = bacc.Bacc("TRN2", target_bir_lowering=False, debug=False)

    batT = nc.declare_dram_parameter("batt", [DIM, N], BF16, isOutput=False)
    qrT = nc.declare_dram_parameter("qrt", [DIM, NQQ], BF16, isOutput=False)
    wqkv = nc.declare_dram_parameter("wqkv", [DIM, 3 * DIM], BF16, isOutput=False)
    wproj = nc.declare_dram_parameter("wproj", [DIM, DIM], BF16, isOutput=False)
    maskT = nc.declare_dram_parameter("maskt", [N, NQQ], FP8, isOutput=False)
    maskb = nc.declare_dram_parameter("maskb", [N, NQQ], BF16, isOutput=False)
    eyeb = nc.declare_dram_parameter("eyeb", [128, 128], FP8, isOutput=False)
    out = nc.declare_dram_parameter("out", [NQQ, DIM], F32, isOutput=True)

    Exp = mybir.ActivationFunctionType.Exp
    ADD = mybir.AluOpType.add
    MULT = mybir.AluOpType.mult

    with tile.TileContext(nc) as tc, ExitStack() as ctx:
        persist = ctx.enter_context(tc.tile_pool(name="persist", bufs=1))
        bpool = ctx.enter_context(tc.tile_pool(name="bpool", bufs=1))
        mpool = ctx.enter_context(tc.tile_pool(name="mpool", bufs=1))
        esbp = ctx.enter_context(tc.tile_pool(name="esbp", bufs=8))
        small = ctx.enter_context(tc.tile_pool(name="small", bufs=4))
        outp = ctx.enter_context(tc.tile_pool(name="outp", bufs=2))

        # ---- persistent loads (order matters: compute-critical first) ----
        wq_sb = []
        for k in range(4):
            t = persist.tile([128, 3 * DIM], BF16, tag=f"wqkv{k}")
            nc.sync.dma_start(out=t, in_=wqkv[k * 128:(k + 1) * 128, :])
            wq_sb.append(t)
        batT_sb = [persist.tile([128, N], BF16, tag=f"batT{k}", name=f"batT{k}")
                   for k in range(4)]
        # column-chunked in k-major order so make_kt_chunk(jc=0) (which needs
        # all 4 k-tiles but only columns 0:512) starts ~8us earlier
        for cc in range(4):
            for k in range(4):
                nc.sync.dma_start(
                    out=batT_sb[k][:, cc * 1024:(cc + 1) * 1024],
                    in_=batT[k * 128:(k + 1) * 128, cc * 1024:(cc + 1) * 1024],
                )
        qrT_sb = []
        for k in range(4):
            t = persist.tile([128, NQQ], BF16, tag=f"qrT{k}")
            nc.sync.dma_start(out=t, in_=qrT[k * 128:(k + 1) * 128, :])
            qrT_sb.append(t)
        eye_sb = persist.tile([128, 128], FP8, tag="eyeb")
        nc.sync.dma_start(out=eye_sb, in_=eyeb[:, :])
        bias_sb = persist.tile([128, 1], F32, tag="biasneg")
        nc.vector.memset(bias_sb, BIAS_NEG)
        wp_sb = []
        for k in range(4):
            t = persist.tile([128, DIM], BF16, tag=f"wproj{k}")
            nc.sync.dma_start(out=t, in_=wproj[k * 128:(k + 1) * 128, :])
            wp_sb.append(t)

        def make_kt_chunk(pool, t, g, jc):
            ps = pool.tile([128, 512], F32, tag="xps")
            for k in range(4):
                nc.tensor.matmul(
                    ps,
                    wq_sb[k][:, DIM + 128 * g: DIM + 128 * g + 128],
                    batT_sb[k][:, jc * 512:(jc + 1) * 512],
                    start=(k == 0), stop=(k == 3),
                )
            nc.scalar.copy(t[:, jc * 512:(jc + 1) * 512], ps)

        def make_qt(pool, t, g, ic):
            ps = pool.tile([128, 512], F32, tag="xps")
            for k in range(4):
                nc.tensor.matmul(
                    ps,
                    wq_sb[k][:, 128 * g: 128 * g + 128],
                    qrT_sb[k][:, ic * 512:(ic + 1) * 512],
                    start=(k == 0), stop=(k == 3),
                )
            nc.scalar.copy(t, ps)

        def make_v(pool, t, nb):
            ps = pool.tile([128, 512], F32, tag="xps")
            for k in range(4):
                nc.tensor.matmul(
                    ps,
                    batT_sb[k][:, nb * 128:(nb + 1) * 128],
                    wq_sb[k][:, 2 * DIM: 3 * DIM],
                    start=(k == 0), stop=(k == 3),
                )
            dst = bass.AP(
                tensor=t.tensor, offset=t.offset,
                ap=[t.ap[0], [33, H], [1, D]],
            )
            nc.scalar.copy(dst, ps)
            ones = bass.AP(
                tensor=t.tensor, offset=t.offset + D,
                ap=[t.ap[0], [33, H]],
            )
            nc.vector.memset(ones, 1.0)

        kt_sb = [bpool.tile([128, N], BF16, tag=f"kt{g}", name=f"kt{g}")
                 for g in range(G)]
        qt_sb = [[bpool.tile([128, NQ], BF16, tag=f"qt{g}i{ic}", name=f"qt{g}i{ic}")
                  for ic in range(2)] for g in range(G)]
        v_sb = [bpool.tile([128, H * (D + 1)], BF16, tag=f"v{nb}", name=f"v{nb}")
                for nb in range(JB)]

        def dispatch(pool, calls, ic):
            for c in calls:
                if c[0] == "v":
                    make_v(pool, v_sb[c[1]], c[1])
                elif c[0] == "kt":
                    make_kt_chunk(pool, kt_sb[c[1]], c[1], c[2])
                else:
                    make_qt(pool, qt_sb[c[1]][c[2]], c[1], c[2])

        for ic in range(2):
            if ic == 0:
                # ---- QKV pre-phase: kt0, qt0(ic0), v0 -------------------
                with tc.tile_pool(name="mm0", bufs=2, space="PSUM") as mm_ps:
                    for jc in range(N // 512):
                        make_kt_chunk(mm_ps, kt_sb[0], 0, jc)
                    make_qt(mm_ps, qt_sb[0][0], 0, 0)
                    make_v(mm_ps, v_sb[0], 0)

            # mask tiles for this pass ([j, i] layout). R1-routed jbs keep a
            # bf16 copy (DVE 2x mask-mul needs 2-byte dtype); rest are fp8.
            mask_sb = []
            for jb in range(JB):
                if jb % 8 in (2, 6):
                    t = mpool.tile([128, NQ], BF16, tag=f"maskb{jb}")
                    src = maskb
                else:
                    t = mpool.tile([128, NQ], FP8, tag=f"mask{jb}")
                    src = maskT
                nc.sync.dma_start(
                    out=t,
                    in_=src[jb * 128:(jb + 1) * 128, ic * 512:(ic + 1) * 512],
                )
                mask_sb.append(t)

            # ---- attention ----------------------------------------------
            # ic0: 6 st banks + 1 av + 1 deferred-QKV scratch = 8
            # ic1: no deferred work -> use the spare bank to double-buffer av
            # so the normalization chain overlaps the next (g,h2) accumulation
            with (tc.tile_pool(name=f"st{ic}", bufs=3, space="PSUM") as st_ps,
                  tc.tile_pool(name=f"avp{ic}", bufs=(1 if ic == 0 else 2),
                               space="PSUM") as av_ps,
                  tc.tile_pool(name=f"xtr{ic}", bufs=1, space="PSUM") if ic == 0
                  else _nullctx() as xtr_ps):
                pre_sb = []

                def slot(g, h2, jb, av):
                    if True:
                        if True:
                            calls = _deferred(ic, g, h2, jb)
                            heavy = bool(calls)
                            # three masked-exp routes, interleaved by jb so
                            # ACT, DVE and PE all stay engaged:
                            #   R3 (jb%8 in 0,4): PE mask-inject + ACT exp
                            #   R1 (jb%8 in 2,6): ACT exp + DVE 2x mask-mul
                            #   R2 (odd jb):      DVE fused Schraudolph STT
                            # heavy slots (deferred QKV on PE) never take R3.
                            m8 = jb % 8
                            r1 = m8 in (2, 6)
                            r3 = (not heavy) and m8 in (0, 4)
                            st = st_ps.tile([128, 1024], F32, tag="st")
                            for rr in range(2):
                                r = 2 * h2 + rr
                                nc.tensor.matmul(
                                    st[:, rr * 512:rr * 512 + 512],
                                    kt_sb[g][32 * r:32 * r + 32, jb * 128:(jb + 1) * 128],
                                    qt_sb[g][ic][32 * r:32 * r + 32, :],
                                    start=True, stop=not r3,
                                    tile_position=(32 * r, 0),
                                )
                            if heavy:
                                dispatch(xtr_ps, calls, ic)
                            e = esbp.tile([128, 1024], BF16, tag="e")
                            if r3:
                                # inject +BIG*mask into both banks
                                for rr in range(2):
                                    nc.tensor.matmul(
                                        st[:, rr * 512:rr * 512 + 512],
                                        eye_sb,
                                        mask_sb[jb],
                                        start=False, stop=True,
                                        skip_group_check=True,
                                    )
                                nc.scalar.activation(e, st, Exp, scale=SC, bias=bias_sb)
                            else:
                                mrep = bass.AP(
                                    tensor=mask_sb[jb].tensor, offset=mask_sb[jb].offset,
                                    ap=[mask_sb[jb].ap[0], [0, 2], [1, 512]],
                                )
                                if r1:
                                    nc.scalar.activation(e, st, Exp, scale=SC)
                                    nc.vector.tensor_mul(e, e, mrep)
                                else:
                                    nc.vector.scalar_tensor_tensor(
                                        out=e.bitcast(I16), in0=st, scalar=B_CONST,
                                        in1=mrep, op0=ADD, op1=MULT,
                                    )
                            for rr in range(2):
                                r = 2 * h2 + rr
                                h = g * HG + r
                                nc.tensor.matmul(
                                    av[64 * rr:64 * rr + 33, 0:512],
                                    v_sb[jb][:, 33 * h: 33 * h + 33],
                                    e[:, rr * 512:rr * 512 + 512],
                                    start=(jb == 0), stop=(jb == JB - 1),
                                    tile_position=(0, 64 * rr),
                                )

                def norm(h2, av, pre):
                    # normalize this h2's two heads -> pre^T rows; frees av
                    for rr in range(2):
                        r = 2 * h2 + rr
                        pb = 64 * rr
                        rsr = small.tile([1, NQ], F32, tag="rsr")
                        nc.scalar.copy(rsr, av[pb + 32: pb + 33, 0:512])
                        rcp = small.tile([1, NQ], F32, tag="rcp")
                        nc.vector.reciprocal_approx_fast(rcp, rsr)
                        rcpb = small.tile([32, NQ], F32, tag="rcpb")
                        nc.gpsimd.partition_broadcast(rcpb, rcp[0:1, :], channels=32)
                        nc.vector.tensor_mul(
                            pre[32 * r: 32 * r + 32, :],
                            av[pb: pb + 32, 0:512],
                            rcpb,
                        )

                for g in range(G):
                    pre = bpool.tile([128, NQ], BF16, tag=f"pre{g}")
                    for h2 in range(2):
                        av = av_ps.tile([128, 512], F32, tag="av",
                                        name=f"av{ic}{g}{h2}")
                        for jb in range(JB):
                            slot(g, h2, jb, av)
                        norm(h2, av, pre)
                    pre_sb.append(pre)

            # ---- output projection --------------------------------------
            with tc.tile_pool(name=f"pj{ic}", bufs=2, space="PSUM") as mm_ps:
                for ib in range(NQ // 128):
                    ps = mm_ps.tile([128, DIM], F32)
                    for g in range(G):
                        nc.tensor.matmul(
                            ps,
                            pre_sb[g][:, ib * 128:(ib + 1) * 128],
                            wp_sb[g],
                            start=(g == 0), stop=(g == 3),
                        )
                    o = outp.tile([128, DIM], F32, tag="o")
                    nc.scalar.copy(o, ps)
                    nc.sync.dma_start(
                        out=out[ic * 512 + ib * 128: ic * 512 + (ib + 1) * 128, :],
                        in_=o,
                    )

    nc.compile()
    return nc


def _prep_inputs(batch, w_qkv, w_proj, custom_mask):
    batch = np.asarray(batch, np.float32)
    wqkv_f = np.asarray(w_qkv, np.float32).copy()
    wqkv_f[:, :DIM] *= np.float32(A_PRE)       # fold Schraudolph pre-scale into Q
    wqkv_bf = wqkv_f.astype(NPBF16)
    wproj_bf = np.asarray(w_proj, np.float32).astype(NPBF16)
    m = np.asarray(custom_mask, np.float32)[0, 0]  # [N, N] 0/1
    eye = (np.eye(128, dtype=np.float32) * np.float32(BIG)).astype(NPFP8)
    batTs = [np.ascontiguousarray(batch[b].T).astype(NPBF16) for b in range(B)]
    in_maps = []
    for c in range(NCORES):
        b, q = divmod(c, B * 2)
        b, q = c // 4, c % 4
        rows = slice(q * NQQ, (q + 1) * NQQ)
        qrT = np.ascontiguousarray(batch[b, rows, :].T)
        mT = np.ascontiguousarray(m[rows, :].T)
        in_maps.append({
            "batt": batTs[b], "qrt": qrT.astype(NPBF16), "wqkv": wqkv_bf,
            "wproj": wproj_bf, "maskt": mT.astype(NPFP8),
            "maskb": mT.astype(NPBF16), "eyeb": eye,
        })
    return in_maps


def _run(in_maps, trace=False, **kw):
    if "nc" not in _CACHE:
        _CACHE["nc"] = build_nc()
    return run_bass_kernel_spmd(
        _CACHE["nc"], in_maps, core_ids=list(range(NCORES)), trace=trace, **kw
    )


def kernel(batch, w_qkv, w_proj, custom_mask):
    in_maps = _prep_inputs(batch, w_qkv, w_proj, custom_mask)
    res = _run(in_maps)
    full = np.empty((B, N, DIM), np.float32)
    for c in range(NCORES):
        b, q = c // 4, c % 4
        full[b, q * NQQ:(q + 1) * NQQ, :] = res.results[c]["out"]
    return full


# revision 47
# speedup vs baseline: 1.0162x; 1.0162x over previous
"""Sparse-attention Bass kernel for 8 TRN2 NeuronCores.

Sharding: (batch, query-quarter) parallel. Core c owns batch b=c//4 and query
rows [q*1024, (q+1)*1024) of that batch (q=c%4), processed as two 512-query
passes (ic=0,1). K/V for the core's batch are computed once and reused by
both passes (half the redundant QKV work of pure query-sharding).

Elementwise strategy (the naive version is exp/mask-bound on ACT+DVE):
  * Q weights are pre-scaled on host by A = 2^7*log2(e)*scale, so PSUM scores
    arrive as s' with exp(s*scale) = 2^(s'/128).
  * Route R2 (DVE): one fused scalar_tensor_tensor per tile computes
    (s' + B) * mask -> int16 (Schraudolph bitcast: int16 pattern read as bf16
    is the masked exp). One DVE op replaces ACT-exp + DVE-mask.
  * Route R3 (PE+ACT): a diagonal matmul injects +BIG*mask into the score
    PSUM (eye*BIG as fp8e5 lhsT, mask fp8e5 rhs), then one ACT exp with
    bias=-BIG*ln2/128 finishes the tile (exact fp32 cancellation for kept
    elements, exp(-177) == 0 for masked ones). No DVE op at all.
  R2/R3 alternate on "light" slots (no deferred-QKV PE work) so ACT, DVE and
  PE all pipeline; the Schraudolph constant C is centered (-6.25) so the
  trick's mean inflation (2.82%) cancels against the true-exp route.

st PSUM pool at bufs=3 is critical: it gives PE enough runway that HAM never
re-throttles the clock to 1.2 GHz (measured 792us -> 25us cold time).
"""

import numpy as np
from contextlib import ExitStack

import concourse.bass as bass
import concourse.tile as tile
from concourse import bacc, mybir
from concourse.bass_utils import run_bass_kernel_spmd

BF16 = mybir.dt.bfloat16
F32 = mybir.dt.float32
I16 = mybir.dt.int16
FP8 = mybir.dt.float8e5
NPBF16 = mybir.dt.np(BF16)
NPFP8 = mybir.dt.np(FP8)

B, N, DIM, H, D = 2, 4096, 512, 16, 32
NCORES = 8
NQQ = 1024                  # query rows per core (one batch elem)
NQ = 512                    # query rows per pass
G = 4                       # head groups (4 heads each)
HG = H // G                 # heads per group (4)
JB = N // 128               # key blocks (32)
SCALE = float(D) ** -0.5

# Schraudolph / exp constants
A_PRE = 128.0 * np.log2(np.e) * SCALE          # folded into Q weights on host
C_CENTER = -6.25                                # centers trick vs true exp
B_CONST = float(127.0 * 128.0 + C_CENTER)       # STT additive constant
SC = float(np.float32(np.log(2.0) / 128.0))     # ACT exp scale (undoes A_PRE)
BIG = 32768.0                                   # mask injection magnitude
BIAS_NEG = float(-(np.float32(SC) * np.float32(BIG)))  # exact fp32 cancel

_CACHE = {}


class _nullctx:
    def __enter__(self):
        return None

    def __exit__(self, *a):
        return False


def _deferred(ic, g, h2, jb):
    """QKV work hidden under ic=0 attention. Returns list of call specs."""
    if ic != 0:
        return []
    calls = []
    if g == 0 and h2 == 0 and jb < JB - 1:
        calls.append(("v", jb + 1))
    if h2 == 1 and g in (0, 1) and jb < 8:
        calls.append(("kt", 2 * g + 1, jb))          # kt1 (g0), kt3 (g1)
    if h2 == 0 and g == 1 and jb < 8:
        calls.append(("kt", 2, jb))                  # kt2
    if jb == 10 and (g, h2) in ((0, 1), (1, 0), (1, 1)):
        calls.append(("qt", 2 * g + h2, 0))          # qt1, qt2, qt3 for ic0
    if g == 1 and h2 == 1 and jb in (12, 14, 16, 18):
        calls.append(("qt", (jb - 12) // 2, 1))      # qt0..qt3 for ic1
    return calls


def build_nc():
    nc = bacc.Bacc("TRN2", target_bir_lowering=False, debug=False)

    batT = nc.declare_dram_parameter("batt", [DIM, N], BF16, isOutput=False)
    qrT = nc.declare_dram_parameter("qrt", [DIM, NQQ], BF16, isOutput=False)
    wqkv = nc.declare_dram_parameter("wqkv", [DIM, 3 * DIM], BF16, isOutput=False)
    wproj = nc.declare_dram_parameter("wproj", [DIM, DIM], BF16, isOutput=False)
    maskT = nc.declare_dram_parameter("maskt", [N, NQQ], FP8, isOutput=False)
    maskb = nc.declare_dram_parameter("maskb", [N, NQQ], BF16, isOutput=False)
    eyeb = nc.declare_dram_parameter("eyeb", [128, 128], FP8, isOutput=False)
    out = nc.declare_dram_parameter("out", [NQQ, DIM], F32, isOutput=True)

    Exp = mybir.ActivationFunctionType.Exp
    ADD = mybir.AluOpType.add
    MULT = mybir.AluOpType.mult

    with tile.TileContext(nc) as tc, ExitStack() as ctx:
        persist = ctx.enter_context(tc.tile_pool(name="persist", bufs=1))
        bpool = ctx.enter_context(tc.tile_pool(name="bpool", bufs=1))
        mpool = ctx.enter_context(tc.tile_pool(name="mpool", bufs=1))
        esbp = ctx.enter_context(tc.tile_pool(name="esbp", bufs=8))
        small = ctx.enter_context(tc.tile_pool(name="small", bufs=4))
        outp = ctx.enter_context(tc.tile_pool(name="outp", bufs=2))

        # ---- persistent loads (order matters: compute-critical first) ----
        wq_sb = []
        for k in range(4):
            t = persist.tile([128, 3 * DIM], BF16, tag=f"wqkv{k}")
            nc.sync.dma_start(out=t, in_=wqkv[k * 128:(k + 1) * 128, :])
            wq_sb.append(t)
        batT_sb = [persist.tile([128, N], BF16, tag=f"batT{k}", name=f"batT{k}")
                   for k in range(4)]
        # column-chunked in k-major order so make_kt_chunk(jc=0) (which needs
        # all 4 k-tiles but only columns 0:512) starts ~8us earlier
        for cc in range(4):
            for k in range(4):
                nc.sync.dma_start(
                    out=batT_sb[k][:, cc * 1024:(cc + 1) * 1024],
                    in_=batT[k * 128:(k + 1) * 128, cc * 1024:(cc + 1) * 1024],
                )
        qrT_sb = []
        for k in range(4):
            t = persist.tile([128, NQQ], BF16, tag=f"qrT{k}")
            nc.sync.dma_start(out=t, in_=qrT[k * 128:(k + 1) * 128, :])
            qrT_sb.append(t)
        eye_sb = persist.tile([128, 128], FP8, tag="eyeb")
        nc.sync.dma_start(out=eye_sb, in_=eyeb[:, :])
        bias_sb = persist.tile([128, 1], F32, tag="biasneg")
        nc.vector.memset(bias_sb, BIAS_NEG)
        wp_sb = []
        for k in range(4):
            t = persist.tile([128, DIM], BF16, tag=f"wproj{k}")
            nc.sync.dma_start(out=t, in_=wproj[k * 128:(k + 1) * 128, :])
            wp_sb.append(t)

        def make_kt_chunk(pool, t, g, jc):
            ps = pool.tile([128, 512], F32, tag="xps")
            for k in range(4):
                nc.tensor.matmul(
                    ps,
                    wq_sb[k][:, DIM + 128 * g: DIM + 128 * g + 128],
                    batT_sb[k][:, jc * 512:(jc + 1) * 512],
                    start=(k == 0), stop=(k == 3),
                )
            nc.scalar.copy(t[:, jc * 512:(jc + 1) * 512], ps)

        def make_qt(pool, t, g, ic):
            ps = pool.tile([128, 512], F32, tag="xps")
            for k in range(4):
                nc.tensor.matmul(
                    ps,
                    wq_sb[k][:, 128 * g: 128 * g + 128],
                    qrT_sb[k][:, ic * 512:(ic + 1) * 512],
                    start=(k == 0), stop=(k == 3),
                )
            nc.scalar.copy(t, ps)

        def make_v(pool, t, nb):
            ps = pool.tile([128, 512], F32, tag="xps")
            for k in range(4):
                nc.tensor.matmul(
                    ps,
                    batT_sb[k][:, nb * 128:(nb + 1) * 128],
                    wq_sb[k][:, 2 * DIM: 3 * DIM],
                    start=(k == 0), stop=(k == 3),
                )
            dst = bass.AP(
                tensor=t.tensor, offset=t.offset,
                ap=[t.ap[0], [33, H], [1, D]],
            )
            nc.scalar.copy(dst, ps)
            ones = bass.AP(
                tensor=t.tensor, offset=t.offset + D,
                ap=[t.ap[0], [33, H]],
            )
            nc.vector.memset(ones, 1.0)

        kt_sb = [bpool.tile([128, N], BF16, tag=f"kt{g}", name=f"kt{g}")
                 for g in range(G)]
        qt_sb = [[bpool.tile([128, NQ], BF16, tag=f"qt{g}i{ic}", name=f"qt{g}i{ic}")
                  for ic in range(2)] for g in range(G)]
        v_sb = [bpool.tile([128, H * (D + 1)], BF16, tag=f"v{nb}", name=f"v{nb}")
                for nb in range(JB)]

        def dispatch(pool, calls, ic):
            for c in calls:
                if c[0] == "v":
                    make_v(pool, v_sb[c[1]], c[1])
                elif c[0] == "kt":
                    make_kt_chunk(pool, kt_sb[c[1]], c[1], c[2])
                else:
                    make_qt(pool, qt_sb[c[1]][c[2]], c[1], c[2])

        for ic in range(2):
            if ic == 0:
                # ---- QKV pre-phase: kt0, qt0(ic0), v0 -------------------
                with tc.tile_pool(name="mm0", bufs=2, space="PSUM") as mm_ps:
                    for jc in range(N // 512):
                        make_kt_chunk(mm_ps, kt_sb[0], 0, jc)
                    make_qt(mm_ps, qt_sb[0][0], 0, 0)
                    make_v(mm_ps, v_sb[0], 0)

            # mask tiles for this pass ([j, i] layout). R1-routed jbs keep a
            # bf16 copy (DVE 2x mask-mul needs 2-byte dtype); rest are fp8.
            mask_sb = []
            for jb in range(JB):
                # ic1's first tiles get fresh buffers so their DMA issues
                # during ic0 instead of stalling on ic0's last mask readers
                sfx = "x" if (ic == 1 and jb < 6) else ""
                if jb % 8 in (2, 6):
                    t = mpool.tile([128, NQ], BF16, tag=f"maskb{jb}{sfx}")
                    src = maskb
                else:
                    t = mpool.tile([128, NQ], FP8, tag=f"mask{jb}{sfx}")
                    src = maskT
                nc.sync.dma_start(
                    out=t,
                    in_=src[jb * 128:(jb + 1) * 128, ic * 512:(ic + 1) * 512],
                )
                mask_sb.append(t)

            # ---- attention ----------------------------------------------
            # 6 st banks + 2 for av/deferred-QKV scratch = 8. The scratch is
            # only needed while deferred QKV runs (ic0, g<2); afterwards its
            # bank double-buffers av so the normalization chain overlaps the
            # next (g,h2) accumulation.
            with tc.tile_pool(name=f"st{ic}", bufs=3, space="PSUM") as st_ps:
                pre_sb = []

                def slot(g, h2, jb, av, xtr_ps):
                    if True:
                        if True:
                            calls = _deferred(ic, g, h2, jb)
                            heavy = bool(calls)
                            # three masked-exp routes, interleaved by jb so
                            # ACT, DVE and PE all stay engaged:
                            #   R3 (jb%8 in 0,4): PE mask-inject + ACT exp
                            #   R1 (jb%8 in 2,6): ACT exp + DVE 2x mask-mul
                            #   R2 (odd jb):      DVE fused Schraudolph STT
                            # heavy slots (deferred QKV on PE) never take R3.
                            m8 = jb % 8
                            r1 = m8 in (2, 6)
                            r3 = (not heavy) and m8 in (0, 4)
                            st = st_ps.tile([128, 1024], F32, tag="st")
                            for rr in range(2):
                                r = 2 * h2 + rr
                                nc.tensor.matmul(
                                    st[:, rr * 512:rr * 512 + 512],
                                    kt_sb[g][32 * r:32 * r + 32, jb * 128:(jb + 1) * 128],
                                    qt_sb[g][ic][32 * r:32 * r + 32, :],
                                    start=True, stop=not r3,
                                    tile_position=(32 * r, 0),
                                )
                            if heavy:
                                dispatch(xtr_ps, calls, ic)
                            e = esbp.tile([128, 1024], BF16, tag="e")
                            if r3:
                                # inject +BIG*mask into both banks
                                for rr in range(2):
                                    nc.tensor.matmul(
                                        st[:, rr * 512:rr * 512 + 512],
                                        eye_sb,
                                        mask_sb[jb],
                                        start=False, stop=True,
                                        skip_group_check=True,
                                    )
                                nc.scalar.activation(e, st, Exp, scale=SC, bias=bias_sb)
                            else:
                                mrep = bass.AP(
                                    tensor=mask_sb[jb].tensor, offset=mask_sb[jb].offset,
                                    ap=[mask_sb[jb].ap[0], [0, 2], [1, 512]],
                                )
                                if r1:
                                    nc.scalar.activation(e, st, Exp, scale=SC)
                                    nc.vector.tensor_mul(e, e, mrep)
                                else:
                                    nc.vector.scalar_tensor_tensor(
                                        out=e.bitcast(I16), in0=st, scalar=B_CONST,
                                        in1=mrep, op0=ADD, op1=MULT,
                                    )
                            for rr in range(2):
                                r = 2 * h2 + rr
                                h = g * HG + r
                                nc.tensor.matmul(
                                    av[64 * rr:64 * rr + 33, 0:512],
                                    v_sb[jb][:, 33 * h: 33 * h + 33],
                                    e[:, rr * 512:rr * 512 + 512],
                                    start=(jb == 0), stop=(jb == JB - 1),
                                    tile_position=(0, 64 * rr),
                                )

                def norm(h2, av, pre):
                    # normalize this h2's two heads -> pre^T rows; frees av
                    for rr in range(2):
                        r = 2 * h2 + rr
                        pb = 64 * rr
                        rsr = small.tile([1, NQ], F32, tag="rsr")
                        nc.scalar.copy(rsr, av[pb + 32: pb + 33, 0:512])
                        rcp = small.tile([1, NQ], F32, tag="rcp")
                        nc.vector.reciprocal_approx_fast(rcp, rsr)
                        rcpb = small.tile([32, NQ], F32, tag="rcpb")
                        nc.gpsimd.partition_broadcast(rcpb, rcp[0:1, :], channels=32)
                        nc.vector.tensor_mul(
                            pre[32 * r: 32 * r + 32, :],
                            av[pb: pb + 32, 0:512],
                            rcpb,
                        )

                def run_half(gs, av_ps, xtr_ps):
                    for g in gs:
                        pre = bpool.tile([128, NQ], BF16, tag=f"pre{g}",
                                         name=f"pre{g}")
                        for h2 in range(2):
                            av = av_ps.tile([128, 512], F32, tag="av",
                                            name=f"av{ic}{g}{h2}")
                            for jb in range(JB):
                                slot(g, h2, jb, av, xtr_ps)
                            norm(h2, av, pre)
                        pre_sb.append(pre)

                if ic == 0:
                    with (tc.tile_pool(name="avA", bufs=1, space="PSUM") as ava,
                          tc.tile_pool(name="xtr0", bufs=1, space="PSUM") as xps):
                        run_half((0, 1), ava, xps)
                    with tc.tile_pool(name="avB", bufs=2, space="PSUM") as avb:
                        run_half((2, 3), avb, None)
                else:
                    with tc.tile_pool(name="avC", bufs=2, space="PSUM") as avc:
                        run_half(range(G), avc, None)

            # ---- output projection --------------------------------------
            with tc.tile_pool(name=f"pj{ic}", bufs=2, space="PSUM") as mm_ps:
                for ib in range(NQ // 128):
                    ps = mm_ps.tile([128, DIM], F32)
                    for g in range(G):
                        nc.tensor.matmul(
                            ps,
                            pre_sb[g][:, ib * 128:(ib + 1) * 128],
                            wp_sb[g],
                            start=(g == 0), stop=(g == 3),
                        )
                    o = outp.tile([128, DIM], F32, tag="o")
                    nc.scalar.copy(o, ps)
                    nc.sync.dma_start(
                        out=out[ic * 512 + ib * 128: ic * 512 + (ib + 1) * 128, :],
                        in_=o,
                    )

    nc.compile()
    return nc


def _prep_inputs(batch, w_qkv, w_proj, custom_mask):
    batch = np.asarray(batch, np.float32)
    wqkv_f = np.asarray(w_qkv, np.float32).copy()
    wqkv_f[:, :DIM] *= np.float32(A_PRE)       # fold Schraudolph pre-scale into Q
    wqkv_bf = wqkv_f.astype(NPBF16)
    wproj_bf = np.asarray(w_proj, np.float32).astype(NPBF16)
    m = np.asarray(custom_mask, np.float32)[0, 0]  # [N, N] 0/1
    eye = (np.eye(128, dtype=np.float32) * np.float32(BIG)).astype(NPFP8)
    batTs = [np.ascontiguousarray(batch[b].T).astype(NPBF16) for b in range(B)]
    in_maps = []
    for c in range(NCORES):
        b, q = divmod(c, B * 2)
        b, q = c // 4, c % 4
        rows = slice(q * NQQ, (q + 1) * NQQ)
        qrT = np.ascontiguousarray(batch[b, rows, :].T)
        mT = np.ascontiguousarray(m[rows, :].T)
        in_maps.append({
            "batt": batTs[b], "qrt": qrT.astype(NPBF16), "wqkv": wqkv_bf,
            "wproj": wproj_bf, "maskt": mT.astype(NPFP8),
            "maskb": mT.astype(NPBF16), "eyeb": eye,
        })
    return in_maps


def _run(in_maps, trace=False, **kw):
    if "nc" not in _CACHE:
        _CACHE["nc"] = build_nc()
    return run_bass_kernel_spmd(
        _CACHE["nc"], in_maps, core_ids=list(range(NCORES)), trace=trace, **kw
    )


def kernel(batch, w_qkv, w_proj, custom_mask):
    in_maps = _prep_inputs(batch, w_qkv, w_proj, custom_mask)
    res = _run(in_maps)
    full = np.empty((B, N, DIM), np.float32)
    for c in range(NCORES):
        b, q = c // 4, c % 4
        full[b, q * NQQ:(q + 1) * NQQ, :] = res.results[c]["out"]
    return full
